# revision 61
# baseline (speedup 1.0000x reference)
"""Trainium2 Bass kernel for nn_Attention_layer_attention_logits.

Reference computation (B=64, C=8, Lq=128, Lk=128, D=512):
    q = query @ wq.T ; k = key @ wk.T ; v = key @ wv.T
    scores = (q @ k.T) / sqrt(D)            # [B, C, Lq, Lk]
    scores[pad] = -1e9
    sv = max over Lq                        # [B, C, Lk]
    enhanced = sv[..., None] * v.sum(Lk)    # rank-1 rows
    out = layernorm(enhanced)

Algebraic restructure (as the validated baseline):
    scores = scale * query @ (wq.T @ wk) @ key.T   (fold wq/wk into M)
    v.sum(Lk) = (key.sum(Lk)) @ wv.T               (keysum instead of full v)
    layernorm of f*u (rank-1): per-(b,c) stats of u only;
        f = svm / sqrt(svm^2 * var_u + eps); out = f*(u - mean)*gamma (+ beta)

This version restructures the dataflow around the PSUM-evacuation
bottleneck (cost-model timeline 128.9us -> target ~65us):
  - scores are computed TRANSPOSED per channel: lhsT = key^T chunk [d, l],
    rhs = q2^T chunk [d, q] -> PSUM [l, q].  The max over q is then a
    free-axis reduce_max STRAIGHT from PSUM (4 channels per bank) -- no
    scores evacuation, no transpose-back.
  - keysum is NOT fused into the kt evac (accum_out) anymore: per (b, c, dc)
    a 1-column matmul knat^T @ ones lands the exact f32 keysum in a small
    per-batch PSUM tile (1 PE cycle each), evacuated once per batch.  The
    key^T evacs become plain fp16 2-channel-wide copies that hit DVE's
    2x 16-bit mode.
  - the output DRAM tensor is fp16 (host casts back to f32): store DMA
    bytes halve, and the finals evac writes fp16.
  - vsum/LN tails run per batch-PAIR (plus two single-batch groups at the
    end so the drain stays short); finals (K=1 rank-1 matmuls from row
    banks on partitions {0,32,64,96}) interleave into later batches'
    channel loops one batch after their tail.

Sharding: data-parallel over batch B across 8 cores (8 batches each), weights
replicated. kernel() takes FULL inputs and returns the FULL output.
"""

import os
import numpy as np

# Problem dims (hardcoded per the self-contained-kernel contract)
B, C, LQ, LK, D = 64, 8, 128, 128, 512
N_CORES = 8
B_LOC = B // N_CORES          # 8 batches per core
NBC = B_LOC * C               # 64 (b,c) pairs per core
DC = D // 128                 # 4 contraction chunks
GBC = 16                      # max bc per tail group (pair of batches)
EPS = 1e-5

REPEAT = int(os.environ.get("BASS_KERNEL_REPEAT", "1"))
# engine per 2-channel kt evac, string over {v,a} cycled
KT_EV = os.environ.get("BASS_KERNEL_KT_EV", "vvva")
# engine per finals evac, string over {v,a} cycled
UP_EV = os.environ.get("BASS_KERNEL_UP_EV", "av")
STAGE_BUFS = int(os.environ.get("BASS_KERNEL_STAGE_BUFS", "6"))
KTP_BUFS = int(os.environ.get("BASS_KERNEL_KTP_BUFS", "3"))
OUTP_BUFS = int(os.environ.get("BASS_KERNEL_OUTP_BUFS", "8"))
TRPS_BUFS = int(os.environ.get("BASS_KERNEL_TRPS_BUFS", "3"))
SCPS_BUFS = int(os.environ.get("BASS_KERNEL_SCPS_BUFS", "2"))
UPP_BUFS = int(os.environ.get("BASS_KERNEL_UPP_BUFS", "2"))

_CACHE = {}
LAST_RESULTS = None
TRACE = bool(int(os.environ.get("BASS_KERNEL_TRACE", "0")))


def _build(beta_nonzero: bool, scale: float, gamma_ones: bool = True):
    from contextlib import ExitStack

    import concourse.bacc as bacc
    import concourse.bass as bass
    import concourse.tile as tile
    import concourse.mybir as mybir

    f32 = mybir.dt.float32
    f16 = mybir.dt.float16
    i32 = mybir.dt.int32
    Alu = mybir.AluOpType
    Act = mybir.ActivationFunctionType
    X = mybir.AxisListType.X

    nc = bacc.Bacc(
        "TRN2", target_bir_lowering=False, debug=False,
        enable_asserts=False, num_devices=N_CORES,
    )

    query_d = nc.dram_tensor("query", [B_LOC, LQ, D], f32, kind="ExternalInput").ap()
    key_d = nc.dram_tensor("key", [B_LOC, C, LK, D], f32, kind="ExternalInput").ap()
    kpm_d = nc.dram_tensor("kpm", [C, LK], i32, kind="ExternalInput").ap()
    wq_d = nc.dram_tensor("wq", [D, D], f32, kind="ExternalInput").ap()
    wk_d = nc.dram_tensor("wk", [D, D], f32, kind="ExternalInput").ap()
    wv_d = nc.dram_tensor("wv", [D, D], f32, kind="ExternalInput").ap()
    gamma_d = nc.dram_tensor("gamma", [D], f32, kind="ExternalInput").ap()
    beta_d = nc.dram_tensor("beta", [D], f32, kind="ExternalInput").ap()
    ident_d = nc.dram_tensor("ident", [128, 128], f32, kind="ExternalInput").ap()
    out_d = nc.dram_tensor("out", [B_LOC, C, LK, D], f16, kind="ExternalOutput").ap()

    with tile.TileContext(nc) as tc, ExitStack() as ctx:
        pers = ctx.enter_context(tc.tile_pool(name="pers", bufs=1))
        # PSUM pools: every PSUM slot is one full bank; 8 banks total.
        # trps 2 + scps 2 + upp 2 + psksp 1 + tailp 1 = 8.
        trps = ctx.enter_context(
            tc.tile_pool(name="trps", bufs=TRPS_BUFS, space="PSUM"))
        scps = ctx.enter_context(
            tc.tile_pool(name="scps", bufs=SCPS_BUFS, space="PSUM"))
        upp = ctx.enter_context(
            tc.tile_pool(name="upp", bufs=UPP_BUFS, space="PSUM"))
        psksp = ctx.enter_context(
            tc.tile_pool(name="psksp",
                         bufs=int(os.environ.get("BASS_KERNEL_PSKS_BUFS", "1")),
                         space="PSUM"))
        stage = ctx.enter_context(tc.tile_pool(name="stage", bufs=STAGE_BUFS))
        ktp = ctx.enter_context(tc.tile_pool(name="ktp", bufs=KTP_BUFS))
        grp = ctx.enter_context(
            tc.tile_pool(name="grp",
                         bufs=int(os.environ.get("BASS_KERNEL_GRP_BUFS", "4"))))
        outp = ctx.enter_context(tc.tile_pool(name="outp", bufs=OUTP_BUFS))

        # ---- persistent tiles ----
        ident = pers.tile([128, 128], f32, tag="ident")
        nc.scalar.dma_start(out=ident, in_=ident_d)
        identh = pers.tile([128, 128], f16, tag="identh")
        nc.vector.tensor_copy(out=identh, in_=ident)
        onesc = pers.tile([128, 1], f16, tag="onesc")
        nc.vector.memset(onesc, 1.0)
        q2t = pers.tile([128, DC, B_LOC, 128], f16, tag="q2t")   # [d,dc,b,q]
        wvt16 = pers.tile([128, DC, D], f16, tag="wvt16")        # [d, dpc, e]
        ks = pers.tile([128, DC, NBC], f32, tag="ks")            # keysum^T
        sv = pers.tile([128, NBC], f32, tag="sv")                # max_q scoresT
        umb = pers.tile([GBC, LK], f32, tag="umb")               # 1-mask rows
        gamb = pers.tile([GBC, D], f32, tag="gamb")
        nc.scalar.dma_start(
            out=gamb,
            in_=bass.AP(tensor=gamma_d.tensor, offset=gamma_d.offset,
                        ap=[[0, GBC]] + gamma_d.ap),
        )
        # tail groups: three batch-pairs, then two singles so the drain
        # after the loop only carries one small group.
        GROUPS = [(b,) for b in range(B_LOC)]
        # LN tails run per batch-PAIR (vsum/stats/f-chain amortized over 16
        # bc); each pair's rows land in two adjacent fin groups' banks.
        TAILS = [(2 * k, 2 * k + 1) for k in range(B_LOC // 2)]
        last_of = {bs[-1]: g for g, bs in enumerate(TAILS)}

        FIN_BASE = [(0, 0), (0, 8), (0, 16), (0, 24),
                    (32, 0), (32, 8), (32, 16), (32, 24)]
        # finals row banks: group g rows live on partition FIN_BASE[g][0],
        # row i at free-slot FIN_BASE[g][1]+i. K=1 matmuls need
        # base_partition in {0,32,64,96}.
        ft2 = pers.tile([128, 2 * GBC, 128], f16, tag="ft2")
        ub2 = pers.tile([128, 2 * GBC, D], f16, tag="ub2")
        if beta_nonzero:
            # K=2 variant: row at partition base+1 carries (ones, beta)
            for gg, bs in enumerate(GROUPS):
                p0, s0 = FIN_BASE[gg]
                n = len(bs) * C
                nc.vector.memset(ft2[p0 + 1: p0 + 2, s0:s0 + n, :], 1.0)
                nc.gpsimd.dma_start(
                    out=ub2[p0 + 1: p0 + 2, s0:s0 + n, :],
                    in_=bass.AP(tensor=beta_d.tensor, offset=beta_d.offset,
                                ap=[[0, 1], [0, n]] + beta_d.ap),
                )

        # ========== phase A: weights / query prep ==========
        pha = ctx.enter_context(tc.tile_pool(name="pha", bufs=1))

        knats = [None] * B_LOC

        def load_key(b, halves):
            # early batches load in 2 halves so the first transposes start
            # while the rest streams; steady-state batches use one DMA
            # (cheaper SWDGE desc-gen on Pool).
            t = stage.tile([128, C, D], f16, tag="knat")
            hs = C // halves
            for h in range(halves):
                nc.gpsimd.dma_start(
                    out=t[:, h * hs:(h + 1) * hs, :],
                    in_=key_d[b, h * hs:(h + 1) * hs].rearrange(
                        "c l d -> l c d"))
            knats[b] = t

        # SWDGE order matches the PE program: query first (the qt
        # transposes are the PE's first work), then key 0, then the weights
        # (prep_m runs as part of `mid`), key 1, and the remaining keys.
        qnat = pha.tile([128, B_LOC, D], f16, tag="qnat")
        nc.gpsimd.dma_start(out=qnat[:, 0:4, :],
                            in_=query_d[0:4].rearrange("b q d -> q b d"))
        nc.gpsimd.dma_start(out=qnat[:, 4:8, :],
                            in_=query_d[4:8].rearrange("b q d -> q b d"))
        load_key(0, 2)
        wqs = pha.tile([128, DC, D], f16, tag="wqs")
        nc.gpsimd.dma_start(out=wqs, in_=wq_d.rearrange("(ec p) d -> p ec d", p=128))
        wks = pha.tile([128, DC, D], f16, tag="wks")
        nc.gpsimd.dma_start(out=wks, in_=wk_d.rearrange("(ec p) d -> p ec d", p=128))
        load_key(1, 2)
        # all remaining keys up front: with 8 stage buffers none of these
        # SWDGE preps ever parks the Pool sequencer on a WAR wait, and the
        # sv-dependent tail ops emitted later can never delay a key load.
        for _b in range(2, B_LOC):
            load_key(_b, 1)

        # query^T (fp16 transposes, fp16 evac)
        qt = pha.tile([128, DC, B_LOC, 128], f16, tag="qt")

        def prep_qt():
            for b in range(B_LOC):
                tpb = trps.tile([128, DC, 128], f16, tag="tp")
                for dc in range(DC):
                    nc.tensor.transpose(
                        tpb[:, dc, :], qnat[:, b, dc * 128:(dc + 1) * 128], identh
                    )
                nc.scalar.copy(out=qt[:, :, b, :], in_=tpb)

        # M = wq.T @ wk (fp16 in, f32 accum) -> msb fp16 [d(part), dc, d']
        msb = pha.tile([128, DC, D], f16, tag="msb")

        def prep_m():
            for dc in range(DC):
                mp = scps.tile([128, D], f32, tag="sc")
                for ec in range(DC):
                    nc.tensor.matmul(
                        mp, wqs[:, ec, dc * 128:(dc + 1) * 128], wks[:, ec, :],
                        start=(ec == 0), stop=(ec == DC - 1))
                nc.scalar.copy(out=msb[:, dc, :], in_=mp)

        # ---- deferred prep (emitted after batch 0's c-loop):
        # wv / mask / ln scales
        wvs = pha.tile([128, DC, D], f16, tag="wvs")

        def prep_deferred():
            nc.gpsimd.dma_start(
                out=wvs, in_=wv_d.rearrange("(ec p) d -> p ec d", p=128))
            mraw = pha.tile([C, LK], i32, tag="mraw")
            nc.scalar.dma_start(out=mraw, in_=kpm_d)
            mf = pha.tile([C, LK], f32, tag="mf")
            nc.vector.tensor_copy(out=mf, in_=mraw)
            # unmasked-indicator rows (1 - mask), duplicated for the two
            # batches a tail pair covers -- already [c, l]-natural
            nc.vector.tensor_scalar(
                out=umb[0:C], in0=mf, scalar1=-1.0, scalar2=1.0,
                op0=Alu.mult, op1=Alu.add,
            )
            nc.gpsimd.dma_start(out=umb[C:2 * C], in_=umb[0:C])
            # wv^T -> wvt16 (fp16 transposes + evac)
            for dpc in range(DC):
                tpw = trps.tile([128, DC, 128], f16, tag="tp")
                for ec in range(DC):
                    nc.tensor.transpose(
                        tpw[:, ec, :], wvs[:, ec, dpc * 128:(dpc + 1) * 128],
                        identh,
                    )
                nc.scalar.copy(
                    out=wvt16[:, dpc, :].rearrange("p (a b) -> p a b", a=DC),
                    in_=tpw)

        # q2^T[d', (b q)] = M^T-contraction with query^T (fp16, f32 accum);
        # h-major so batches 0-3's chunks (h=0) are all ready first
        def prep_q2():
            for h in range(2):
                for dpc in range(DC):
                    qp = scps.tile([128, D], f32, tag="sc")
                    for dc in range(DC):
                        nc.tensor.matmul(
                            qp, msb[:, dc, dpc * 128:(dpc + 1) * 128],
                            qt[:, dc, h * 4:h * 4 + 4, :].rearrange("p a b -> p (a b)"),
                            start=(dc == 0), stop=(dc == DC - 1))
                    nc.scalar.copy(
                        out=q2t[:, dpc, h * 4:h * 4 + 4, :].rearrange(
                            "p a b -> p (a b)"),
                        in_=qp)

        def prep_pre():
            prep_qt()

        def prep_all():
            prep_m()
            prep_q2()
            prep_deferred()

        # ========================== main loop ===============================
        deferred_stores = []
        defer_stores = [False]
        slot_state = {}
        small_slots = [False]

        def fin_matmuls(g, bi, ch, pools=None):
            """2 rank-1 output matmuls for one store unit of group g."""
            p0, s0 = FIN_BASE[g]
            ups = []
            for cj in range(2):
                i = s0 + bi * C + ch * 2 + cj
                if pools is None:
                    up = upp.tile([128, D], f32, tag="up", name=f"up{cj}")
                else:
                    pool, ptag = pools[(ch * 2 + cj) % len(pools)]
                    up = pool.tile([128, D], f32, tag=ptag, name=f"up{cj}")
                kk = 2 if beta_nonzero else 1
                nc.tensor.matmul(
                    up, ft2[p0:p0 + kk, i, :], ub2[p0:p0 + kk, i, :],
                    start=True, stop=True)
                ups.append(up)
            return (g, bi, ch, ups)

        def fin_store(unit):
            """evacs into a 4-channel slot; DMA fires when the slot's second
            half lands (fewer, bigger stores -> half the HWDGE/sem
            overheads). Units for one batch arrive in ch order."""
            g, bi, ch, ups = unit
            b = GROUPS[g][bi]
            if small_slots[0]:
                # drain: 2-channel slots so the very last store (and its
                # semaphore) clears ~0.7us sooner
                slot = outp.tile([128, 2, D], f16, tag="slot", name="slot")
                for cj in range(2):
                    ev = UP_EV[(b * C + ch * 2 + cj) % len(UP_EV)]
                    if ev == "v":
                        nc.vector.tensor_copy(out=slot[:, cj, :], in_=ups[cj])
                    else:
                        nc.scalar.copy(out=slot[:, cj, :], in_=ups[cj])
                nc.sync.dma_start(
                    out=out_d[b, ch * 2:ch * 2 + 2].rearrange("c l d -> l c d"),
                    in_=slot)
                return
            half = ch % 2
            if half == 0:
                slot_state[b] = outp.tile([128, 4, D], f16, tag="slot",
                                          name="slot")
            slot = slot_state[b]
            for cj in range(2):
                ev = UP_EV[(b * C + ch * 2 + cj) % len(UP_EV)]
                if ev == "v":
                    nc.vector.tensor_copy(out=slot[:, half * 2 + cj, :],
                                          in_=ups[cj])
                else:
                    nc.scalar.copy(out=slot[:, half * 2 + cj, :], in_=ups[cj])
            if half == 1:
                c0 = (ch - 1) * 2
                dst = out_d[b, c0:c0 + 4].rearrange("c l d -> l c d")
                if defer_stores[0]:
                    deferred_stores.append((dst, slot))
                else:
                    nc.sync.dma_start(out=dst, in_=slot)

        def tail_ks(g, pre=False):
            """keysum-side tail for pair g: vsum, stats, ub rows. Emitted
            BEFORE the pair's scores -- everything here depends only on the
            channel loop, so the ub-row DMA beats the output-store pileup
            on the DMA engines."""
            bs = TAILS[g]
            nb = len(bs)
            n = nb * C
            g0 = bs[0] * C
            ks16 = grp.tile([128, DC, GBC], f16, tag="ks16")
            nc.gpsimd.tensor_copy(out=ks16[:, :, :n], in_=ks[:, :, g0:g0 + n])
            # pre-scores (last pair): use the psks bank so the upcoming
            # scores don't wait on this tile's readers for an scps slot
            vt = (psksp if pre else scps).tile(
                [GBC, D], f32, tag="psks" if pre else "sc", name="vt")
            for dc in range(DC):
                nc.tensor.matmul(
                    vt[:n, :], ks16[:, dc, :n], wvt16[:, dc, :],
                    start=(dc == 0), stop=(dc == DC - 1),
                )
            # stats / ubarg read the matmul result straight from PSUM
            stats = grp.tile([GBC, 6], f32, tag="stats")
            nc.vector.bn_stats(out=stats[:n], in_=vt[:n, :])
            mv = grp.tile([GBC, 2], f32, tag="mv")
            nc.vector.bn_aggr(out=mv[:n], in_=stats[:n])
            # LN of the rank-1 rows: f = svm*rsqrt(svm^2 var + eps) ==
            # sign(svm)*rsqrt(var) to ~1e-8 here (svm^2 var >> eps), so the
            # rsqrt(var) scale folds into the ub row as a per-partition
            # scalar and the ft row becomes a bare sign pattern.
            rts = grp.tile([GBC, 1], f32, tag="rts")
            nc.scalar.activation(rts[:n], mv[:n, 1:2], Act.Sqrt)
            rstd = grp.tile([GBC, 1], f32, tag="rstd")
            nc.vector.reciprocal(out=rstd[:n], in_=rts[:n])
            ubarg = grp.tile([GBC, D], f16, tag="ubarg")
            if gamma_ones:
                # ubarg = (vt - mean)*rstd in one Activation op
                negmr = grp.tile([GBC, 1], f32, tag="negmr")
                nc.vector.tensor_tensor(out=negmr[:n], in0=mv[:n, 0:1],
                                        in1=rstd[:n], op=Alu.mult)
                nc.vector.tensor_scalar(
                    out=negmr[:n], in0=negmr[:n], scalar1=-1.0, scalar2=None,
                    op0=Alu.mult)
                nc.scalar.activation(ubarg[:n], vt[:n, :], Act.Identity,
                                     bias=negmr[:n, 0:1], scale=rstd[:n, 0:1])
            else:
                ubg = grp.tile([GBC, D], f32, tag="ubg")
                nc.vector.scalar_tensor_tensor(
                    out=ubg[:n], in0=vt[:n, :], scalar=mv[:n, 0:1],
                    in1=gamb[:n], op0=Alu.subtract, op1=Alu.mult,
                )
                nc.vector.tensor_scalar(
                    out=ubarg[:n], in0=ubg[:n], scalar1=rstd[:n, 0:1],
                    scalar2=None, op0=Alu.mult)
            p0, s0 = FIN_BASE[bs[0]]
            if nb == 2:
                assert FIN_BASE[bs[1]] == (p0, s0 + C)
            # defer the row DMA: an immediate SWDGE prep would park the
            # Pool sequencer on the ubarg wait
            tail_dmas.append((ub2[p0:p0 + 1, s0:s0 + n, :], ubarg[:n]))
            return g, nb, n, g0, p0, s0, mv

        def tail_sv(state):
            """sv-side tail: the ft row is just sign(svm) = masked? -1 :
            (sv>0? +1 : -1) -- a transpose, one compare and one mask op
            (rsqrt(var) was folded into the ub rows). Deferred into the
            next batch's channel loop so no in-order queue head-blocks on
            sv-dependent work."""
            g, nb, n, g0, p0, s0, mv = state

            def stage1():
                svT = scps.tile([GBC, 128], f32, tag="sc")
                nc.tensor.transpose(svT[:n], sv[:, g0:g0 + n], ident)
                a2 = grp.tile([GBC, 128], f32, tag="a2")
                nc.vector.tensor_scalar(
                    out=a2[:n], in0=svT[:n], scalar1=0.0, scalar2=2.0,
                    op0=Alu.is_gt, op1=Alu.mult)
                ax = grp.tile([GBC, 128], f32, tag="ax")
                nc.gpsimd.tensor_tensor(out=ax[:n], in0=a2[:n], in1=umb[:n],
                                        op=Alu.mult)
                ft16 = grp.tile([GBC, 128], f16, tag="ft16")
                nc.gpsimd.tensor_scalar(
                    out=ft16[:n], in0=ax[:n], scalar1=-1.0, scalar2=None,
                    op0=Alu.add)
                nc.gpsimd.dma_start(
                    out=ft2[p0:p0 + 1, s0:s0 + n, :], in_=ft16[:n])

            tail_pes.append(stage1)

        def units_of(g):
            return [(g, bi, ch) for bi in range(len(GROUPS[g]))
                    for ch in range(C // 2)]

        # pair k's tail is emitted after batch 2k+1 and its f-chain/row
        # DMAs run at batch 2k+2 ci==1; the even group's finals go LATE
        # (ci==3) in that same batch, the odd group's spread over batch
        # 2k+3. The last pair's groups drain after the loop.
        fins_at = {}
        for k in range(B_LOC // 2 - 1):
            fins_at[2 * k + 2] = ((), units_of(2 * k))       # late only
            fins_at[2 * k + 3] = (units_of(2 * k + 1), ())   # spread
        drain_units = units_of(B_LOC - 2) + units_of(B_LOC - 1)
        FIN_DEPTH = int(os.environ.get("BASS_KERNEL_FIN_DEPTH", "3"))

        tail_dmas = []
        tail_pes = []
        tail_states = []

        def flush_tail_dmas():
            while tail_dmas:
                dst, src = tail_dmas.pop(0)
                nc.gpsimd.dma_start(out=dst, in_=src)

        def do_b(b, fin_units=((), ()), mid=None, pre_scores=None):
            kt = ktp.tile([128, DC, C, 128], f16, tag="kt")
            psks = psksp.tile([128, DC, C], f32, tag="psks")
            knat = knats[b]
            spread_units, late_units = fin_units
            nu = len(spread_units)
            pending = []

            def push_fin(u, depth=FIN_DEPTH):
                while len(pending) >= depth:
                    fin_store(pending.pop(0))
                pending.append(fin_matmuls(*u))

            for ci in range(4):            # channel pairs
                c = 2 * ci
                tp = trps.tile([128, DC, 2, 128], f16, tag="tp")
                for j in range(2):
                    for dc in range(DC):
                        nc.tensor.transpose(
                            tp[:, dc, j, :],
                            knat[:, c + j, dc * 128:(dc + 1) * 128], identh)
                for j in range(2):
                    for dc in range(DC):
                        nc.tensor.matmul(
                            psks[:, dc, c + j:c + j + 1],
                            knat[:, c + j, dc * 128:(dc + 1) * 128], onesc,
                            start=True, stop=True)
                ev = KT_EV[(b * 4 + ci) % len(KT_EV)]
                if ev == "v":
                    nc.vector.tensor_copy(out=kt[:, :, c:c + 2, :], in_=tp)
                else:
                    nc.scalar.copy(out=kt[:, :, c:c + 2, :], in_=tp)
                if ci == 0:
                    flush_tail_dmas()
                if ci == 1:
                    while tail_pes:
                        tail_pes.pop(0)()
                for u in spread_units[ci * nu // 4:(ci + 1) * nu // 4]:
                    push_fin(u)
                if ci == 3:
                    for u in late_units:
                        push_fin(u)
            knats[b] = None
            # keysum evac, one per batch
            nc.vector.tensor_copy(out=ks[:, :, b * C:(b + 1) * C], in_=psks)
            if mid is not None:
                mid()
            if pre_scores is not None:
                tail_states.append(tail_ks(pre_scores, pre=True))
            # scoresT [l, q] per channel, 4 channels per PSUM bank;
            # max over q straight from PSUM
            for h in range(2):
                sc = scps.tile([128, 4, 128], f32, tag="sc")
                for cj in range(4):
                    cc = h * 4 + cj
                    for dc in range(DC):
                        nc.tensor.matmul(
                            sc[:, cj, :], kt[:, dc, cc, :], q2t[:, dc, b, :],
                            start=(dc == 0), stop=(dc == DC - 1))
                bc0 = b * C + h * 4
                nc.vector.reduce_max(sv[:, bc0:bc0 + 4], sc, axis=X)
            while pending:
                fin_store(pending.pop(0))

        for _rep in range(REPEAT):
            for b in range(B_LOC):
                if knats[b] is None:
                    load_key(b, 2 if b < 2 else 1)
                if _rep == 0 and b == 0:
                    prep_pre()
                mid = prep_all if (_rep == 0 and b == 0) else None
                g = last_of.get(b)
                if b == B_LOC - 1:
                    defer_stores[0] = True
                do_b(b, fin_units=fins_at.get(b, ((), ())), mid=mid)
                if g is not None:
                    tail_sv(tail_ks(g))
            # drain: the final pair's finals. The main-loop PSUM pools
            # are idle by now, so spread the up tiles across them.
            while tail_pes:
                tail_pes.pop(0)()
            flush_tail_dmas()
            defer_stores[0] = False
            while deferred_stores:
                dst, slot = deferred_stores.pop(0)
                nc.sync.dma_start(out=dst, in_=slot)
            drain_pools = [(trps, "tp"), (scps, "sc"), (upp, "up")]
            small_slots[0] = True
            pend = []
            for u in drain_units:
                while len(pend) >= FIN_DEPTH:
                    fin_store(pend.pop(0))
                pend.append(fin_matmuls(*u, pools=drain_pools))
            while pend:
                fin_store(pend.pop(0))
            small_slots[0] = False

    nc.compile()
    return nc


def _get_nc(beta_nonzero: bool, scale: float, gamma_ones: bool = True):
    key = (beta_nonzero, gamma_ones, KT_EV, UP_EV, REPEAT,
           STAGE_BUFS, KTP_BUFS, OUTP_BUFS, TRPS_BUFS, SCPS_BUFS, UPP_BUFS)
    if key not in _CACHE:
        _CACHE[key] = _build(beta_nonzero, scale, gamma_ones)
    return _CACHE[key]


def kernel(query, key, key_padding_mask, wq, wk, wv, ln_gamma, ln_beta):
    global LAST_RESULTS
    from concourse.bass_utils import run_bass_kernel_spmd

    query = np.ascontiguousarray(np.asarray(query, dtype=np.float32))
    key = np.ascontiguousarray(np.asarray(key, dtype=np.float32))
    kpm = np.ascontiguousarray(np.asarray(key_padding_mask).astype(np.int32))
    wq = np.ascontiguousarray(np.asarray(wq, dtype=np.float32))
    wk = np.ascontiguousarray(np.asarray(wk, dtype=np.float32))
    wv = np.ascontiguousarray(np.asarray(wv, dtype=np.float32))
    gamma = np.ascontiguousarray(np.asarray(ln_gamma, dtype=np.float32))
    beta = np.ascontiguousarray(np.asarray(ln_beta, dtype=np.float32))
    ident = np.eye(128, dtype=np.float32)

    scale = float(1.0 / np.sqrt(np.float32(D)))
    beta_nonzero = bool(np.any(beta != 0.0))
    gamma_ones = bool(np.all(gamma == 1.0))
    nc = _get_nc(beta_nonzero, scale, gamma_ones)

    in_maps = []
    for i in range(N_CORES):
        sl = slice(i * B_LOC, (i + 1) * B_LOC)
        in_maps.append({
            "query": np.ascontiguousarray(query[sl]),
            "key": np.ascontiguousarray(key[sl]),
            "kpm": kpm,
            "wq": wq, "wk": wk, "wv": wv,
            "gamma": gamma, "beta": beta,
            "ident": ident,
        })

    res = run_bass_kernel_spmd(
        nc, in_maps, core_ids=list(range(N_CORES)), trace=TRACE,
    )
    LAST_RESULTS = res
    out = np.concatenate([r["out"] for r in res.results], axis=0)
    return out.astype(np.float32)
